# revision 31
# baseline (speedup 1.0000x reference)
"""Trainium2 Bass kernel for CBSA (cross-block self-attention) module.

Shapes (hardcoded from the problem spec):
  x: [8, 4096, 512], proj_w/to_out_w: [512, 512], step_rep/step_x: [8,1,1],
  to_out_b: [512].  Output: [8, 4096, 512].

Sharding: data-parallel over batch, 1 batch per NeuronCore (8 cores).

Structure:
  - pooling is linear and commutes with the proj GEMM, so pooled x is
    computed on host and rep^T comes from a tiny fp8 on-device GEMM.
  - P1 streams x^T (fp8) in 8 chunks; per chunk: fp8-DoubleRow wT GEMM,
    dots (block-diag rep lhsT), exp -> ed (fp8), and packed transposes:
    a bf16 [128,128] transpose moves a PAIR of adjacent fp8 n-columns,
    halving transpose count; copies run at 2-byte DVE speed.
  - rep_delta contracts the packed pairs via two stride-2 byte-plane fp8
    matmuls per tile; an extra fp8(1,1) column in the packed w tiles
    makes the same matmuls accumulate the softmax row-sums s1 for free.
  - per-pair tail (pairs interleaved step-wise to keep PE fed):
    rep update, pooled self-attention, xds^T computed directly as
    rnat^T @ ed2 (ed2 is symmetric), M = xds^T' @ to_out^T with the
    1/(s1*s2) scale folded into the M copy.
  - P5: out^T[d, n] accumulated over pair-groups with fp8-DoubleRow
    matmuls; bias is a per-partition scalar fused into the PSUM->SBUF
    copy; output is bf16 [C, N], host transposes/casts back.
"""

from contextlib import ExitStack

import numpy as np
import ml_dtypes

import concourse.bass as bass
import concourse.tile as tile
from concourse import bacc, mybir
from concourse import bass_utils

F32 = mybir.dt.float32
BF16 = mybir.dt.bfloat16
FP8 = mybir.dt.float8e4

B = 8
N = 4096
C = 512
HEADS = 8
DH = 64
Q = 64            # pooled tokens
SCALE = DH ** -0.5
NT = N // 128     # 32 token tiles
CH = C // 128     # 4 feature chunks
PAIRS = HEADS // 2  # 4 head pairs
NS = N // 512     # 8 free-dim slices of 512
NCHUNK = 8        # x streamed in 8 chunks of 512 tokens

_CACHE = {}


def _build():
    nc = bacc.Bacc("TRN2", target_bir_lowering=False, debug=False, num_devices=B)

    xT_d = nc.dram_tensor("xT", [128, 2, 2, N], FP8, kind="ExternalInput").ap()
    pwT8_d = nc.dram_tensor("pwT8", [128, 2, 2, C], FP8, kind="ExternalInput").ap()
    dblk_d = nc.dram_tensor("dblk8", [128, PAIRS, 128], FP8,
                            kind="ExternalInput").ap()
    rpT_d = nc.dram_tensor("rpTb", [128, PAIRS, 128], BF16,
                           kind="ExternalInput").ap()
    twT_d = nc.dram_tensor("twT", [128, CH, C], BF16, kind="ExternalInput").ap()
    biasT_d = nc.dram_tensor("biasT", [128, CH], F32, kind="ExternalInput").ap()
    srep_d = nc.dram_tensor("srep", [128, PAIRS], F32, kind="ExternalInput").ap()
    idb_d = nc.dram_tensor("identb", [128, 128], BF16, kind="ExternalInput").ap()
    out_d = nc.dram_tensor("out", [C, N], BF16, kind="ExternalOutput").ap()

    with tile.TileContext(nc) as tc:
        with ExitStack() as ctx:
            _body.ctx = ctx
            _body(tc, nc, xT_d, pwT8_d, dblk_d, rpT_d, twT_d, biasT_d,
                  srep_d, idb_d, out_d)
    nc.compile()
    return nc


def _body(tc, nc, xT_d, pwT8_d, dblk_d, rpT_d, twT_d, biasT_d, srep_d,
          idb_d, out_d):
    Exp = mybir.ActivationFunctionType.Exp
    Ident = mybir.ActivationFunctionType.Identity
    X = mybir.AxisListType.X
    ADD = mybir.AluOpType.add
    MULT = mybir.AluOpType.mult

    ctx = _body.ctx
    const = ctx.enter_context(tc.tile_pool(name="const", bufs=1))
    persist = ctx.enter_context(tc.tile_pool(name="persist", bufs=1))
    xs_pool = ctx.enter_context(tc.tile_pool(name="xstream", bufs=2))
    sm_pool = ctx.enter_context(tc.tile_pool(name="small", bufs=2))
    ost_pool = ctx.enter_context(tc.tile_pool(name="ostage", bufs=2))

    # ---- constants / small inputs (issued before the big x stream) ----
    pwT8 = const.tile([128, 2, 2, C], FP8, tag="pwT8")
    nc.sync.dma_start(pwT8[:], pwT8_d[:])
    dblk_t = const.tile([128, PAIRS, 128], FP8, tag="dblk_t")
    nc.sync.dma_start(dblk_t[:], dblk_d[:])
    # prefetch first x chunks ahead of the remaining constants
    cols = N // NCHUNK
    xts_list = []
    for ck in range(NCHUNK):
        xts = xs_pool.tile([128, 2, 2, cols], FP8, tag="xs", name=f"xts{ck}")
        xts_list.append(xts)
    for ck in range(3):
        nc.sync.dma_start(xts_list[ck][:],
                          xT_d[:, :, :, ck * cols:(ck + 1) * cols])
    identb = const.tile([128, 128], BF16, tag="identb")
    nc.sync.dma_start(identb[:], idb_d[:])
    rpT_t = const.tile([128, PAIRS, 128], BF16, tag="rpT_t")
    nc.sync.dma_start(rpT_t[:], rpT_d[:])
    srep = const.tile([128, PAIRS], F32, tag="srep")
    nc.sync.dma_start(srep[:], srep_d[:])
    twT = const.tile([128, CH, C], BF16, tag="twT")
    nc.sync.dma_start(twT[:], twT_d[:])
    biasT = const.tile([128, CH], F32, tag="biasT")
    nc.sync.dma_start(biasT[:], biasT_d[:])

    # ---- persistent intermediates ----
    wtb = [persist.tile([128, N], FP8, tag=f"wtb{di}", name=f"wtb{di}")
           for di in range(CH)]
    # packed w natural: bf16 element (n2, d) = fp8 pair (w[2*n2,d], w[2*n2+1,d])
    # per (tile, pair) block of 129 cols: [0:128] w-pairs, col 128 = fp8(1,1)
    # so rep_delta's DR matmul also accumulates s1 = sum_n at[n, q] in col 128.
    w_sb = persist.tile([128, NT // 2, PAIRS, 129], BF16, tag="w_sb")
    ed_grp = [persist.tile([128, 2, N], FP8, tag=f"edg{u}", name=f"edg{u}")
              for u in range(PAIRS // 2)]
    # packed attn^T: bf16 element (n2, q) = fp8 pair (ed[q,2*n2], ed[q,2*n2+1])
    at_list = [persist.tile([128, NT // 2, 128], BF16, tag=f"at{p}",
                            name=f"at{p}")
               for p in range(PAIRS)]
    M_grp = [persist.tile([128, 2, C], FP8, tag=f"Mg{u}", name=f"Mg{u}")
             for u in range(PAIRS // 2)]

    with ExitStack() as phase_ctx:
        psB = phase_ctx.enter_context(
            tc.tile_pool(name="psB", bufs=3, space="PSUM"))
        psC = phase_ctx.enter_context(
            tc.tile_pool(name="psC", bufs=2, space="PSUM"))
        psD = phase_ctx.enter_context(
            tc.tile_pool(name="psD", bufs=3, space="PSUM"))

        ONES_PAIR = float(np.frombuffer(bytes([0x38, 0x38]),
                                        ml_dtypes.bfloat16)[0])
        nc.vector.memset(w_sb[:, :, :, 128:129], ONES_PAIR)

        # rep (pooled queries) is computed on host: dblk = 16*repT
        # block-diag per pair (fp8, dots lhsT), rpT = its natural-layout
        # transpose (bf16, rep update operand).
        dblk = [dblk_t[:, p, :] for p in range(PAIRS)]
        rpT = [rpT_t[:, p, :] for p in range(PAIRS)]

        # ============ P1: streamed wT GEMM + dots + exp + transposes ======
        # wT[d, n] = sum_c pw[d, c] x[n, c]; lhsT = pwT chunk, rhs = xT chunk
        Copy = mybir.ActivationFunctionType.Copy
        for ck in range(NCHUNK):
            c0 = ck * cols
            xts = xts_list[ck]
            if ck >= 3:
                nc.sync.dma_start(xts[:], xT_d[:, :, :, c0:c0 + cols])
            # wT slices for this chunk (fp8 DoubleRow)
            for di in range(CH):
                wps = psB.tile([128, 512], F32, tag="psB", name="wps")
                for g in range(2):
                    nc.tensor.matmul(
                        wps[:], pwT8[:, g, :, di * 128:(di + 1) * 128],
                        xts[:, g, :, :],
                        start=(g == 0), stop=(g == 1),
                        perf_mode=mybir.MatmulPerfMode.DoubleRow)
                dst = wtb[di][:, c0:c0 + cols]
                nc.vector.tensor_scalar_mul(dst, wps[:], 1.0 / 16.0)
            # dots + exp for this chunk's 512-slice, all pairs
            for p in range(PAIRS):
                sl = ck
                dps = psB.tile([128, 512], F32, tag="psB", name="dps")
                nc.tensor.matmul(dps[:], dblk[p][:],
                                 wtb[p][:, sl * 512:(sl + 1) * 512],
                                 start=True, stop=True)
                nc.scalar.activation(
                    ed_grp[p // 2][:, p % 2, sl * 512:(sl + 1) * 512],
                    dps[:], Exp, scale=SCALE / 16.0)
            # packed transposes every 2 chunks: bf16 view pairs adjacent
            # fp8 n-columns, so one [128,128] bf16 transpose covers 2 tiles.
            if ck % 2 == 1:
                t4 = ck // 2            # 4 bf16 n2-tiles per 2-chunk group

                def w_tr(di):
                    wtp = psC.tile([128, 4, 128], BF16, tag="psC", name="wtp")
                    wvv = wtb[di].bitcast(BF16)
                    for j in range(4):
                        nc.tensor.transpose(
                            wtp[:, j, :],
                            wvv[:, (t4 * 4 + j) * 128:(t4 * 4 + j + 1) * 128],
                            identb[:])
                    nc.vector.tensor_copy(
                        w_sb[:, t4 * 4:t4 * 4 + 4, di, 0:128], wtp[:])

                def at_tr(p):
                    atp = psC.tile([128, 4, 128], BF16, tag="psC", name="atp")
                    evv = ed_grp[p // 2][:, p % 2, :].bitcast(BF16)
                    for j in range(4):
                        nc.tensor.transpose(
                            atp[:, j, :],
                            evv[:, (t4 * 4 + j) * 128:(t4 * 4 + j + 1) * 128],
                            identb[:])
                    nc.vector.tensor_copy(
                        at_list[p][:, t4 * 4:t4 * 4 + 4, :], atp[:])

                if ck == NCHUNK - 1:
                    # final chunk: interleave so rep_delta(p) unblocks
                    # progressively in pair order
                    for p in range(PAIRS):
                        w_tr(p)
                        at_tr(p)
                else:
                    for di in range(CH):
                        w_tr(di)
                    for p in range(PAIRS):
                        at_tr(p)

        # ============ P4: pooled attention tail (pairs interleaved) ======
        ssc_l, rnat_l, rnT_l, ed2_l, xds_l, xdsT_l = [], [], [], [], [], []
        rdsb_l = []
        for p in range(PAIRS):
            rd_ps = psD.tile([128, 129], F32, tag="psD", name=f"rd{p}")
            for t in range(NT // 2):
                atv = at_list[p][:, t, :].bitcast(FP8)
                wvv = w_sb[:, t, p, :].bitcast(FP8)
                for j in range(2):
                    nc.tensor.matmul(rd_ps[:], atv[:, j:256:2],
                                     wvv[:, j:258:2],
                                     start=(t == 0 and j == 0),
                                     stop=(t == NT // 2 - 1 and j == 1))
            rdsb = sm_pool.tile([128, 129], F32, tag=f"rdsb{p}",
                                name=f"rdsb{p}")
            nc.vector.tensor_copy(rdsb[:], rd_ps[:])
            rdsb_l.append(rdsb)
        for p in range(PAIRS):
            rc1 = sm_pool.tile([128, 1], F32, tag=f"rc1_{p}", name=f"rc1_{p}")
            nc.vector.reciprocal(rc1[:], rdsb_l[p][:, 128:129])
            ssc = sm_pool.tile([128, 1], F32, tag=f"ssc_{p}", name=f"ssc_{p}")
            nc.vector.tensor_mul(ssc[:], rc1[:], srep[:, p:p + 1])
            ssc_l.append((rc1, ssc))
        for p in range(PAIRS):
            rnat = sm_pool.tile([128, 128], BF16, tag=f"rnat{p}",
                                name=f"rnat{p}")
            nc.vector.memset(rnat[:], 0.0)
            for h in range(2):
                r0, r1 = 64 * h, 64 * (h + 1)
                nc.vector.scalar_tensor_tensor(
                    rnat[r0:r1, r0:r1], rdsb_l[p][r0:r1, r0:r1],
                    ssc_l[p][1][r0:r1, 0:1],
                    rpT[p][r0:r1, r0:r1], MULT, ADD)
            rnat_l.append(rnat)
        for p in range(PAIRS):
            rtp = psC.tile([128, 128], BF16, tag="psC", name=f"rtp{p}")
            nc.tensor.transpose(rtp[:], rnat_l[p][:], identb[:])
            rnT = sm_pool.tile([128, 128], BF16, tag=f"rnT{p}", name=f"rnT{p}")
            nc.vector.tensor_copy(rnT[:], rtp[:])
            rnT_l.append(rnT)
        s2_l = []
        for p in range(PAIRS):
            d2_ps = psD.tile([128, 128], F32, tag="psD", name=f"d2{p}")
            nc.tensor.matmul(d2_ps[:], rnT_l[p][:], rnT_l[p][:],
                             start=True, stop=True)
            ed2 = sm_pool.tile([128, 128], BF16, tag=f"ed2_{p}",
                               name=f"ed2_{p}")
            nc.vector.memset(ed2[:], 0.0)
            s2 = sm_pool.tile([128, 1], F32, tag=f"s2_{p}", name=f"s2_{p}")
            for h in range(2):
                r0, r1 = 64 * h, 64 * (h + 1)
                nc.scalar.activation(ed2[r0:r1, r0:r1], d2_ps[r0:r1, r0:r1],
                                     Exp, scale=SCALE,
                                     accum_out=s2[r0:r1, 0:1])
            ed2_l.append(ed2)
            s2_l.append(s2)
        sc_l = []
        for p in range(PAIRS):
            # xds^T[d, q] = sum_k rnat[k, d] * ed2[q, k]  (ed2 symmetric)
            xt_ps = psD.tile([128, 128], F32, tag="psD", name=f"xt{p}")
            nc.tensor.matmul(xt_ps[:], rnat_l[p][:], ed2_l[p][:],
                             start=True, stop=True)
            xdsT = sm_pool.tile([128, 128], BF16, tag=f"xdsT{p}",
                                name=f"xdsT{p}")
            nc.vector.tensor_copy(xdsT[:], xt_ps[:])
            xdsT_l.append(xdsT)
            rc2 = sm_pool.tile([128, 1], F32, tag=f"rc2_{p}", name=f"rc2_{p}")
            nc.vector.reciprocal(rc2[:], s2_l[p][:])
            sc = sm_pool.tile([128, 1], F32, tag=f"sc_{p}", name=f"sc_{p}")
            nc.vector.tensor_mul(sc[:], ssc_l[p][0][:], rc2[:])
            sc_l.append(sc)
        for p in range(PAIRS):
            mp_ps = psB.tile([128, 512], F32, tag="psB", name=f"mp_ps{p}")
            nc.tensor.matmul(mp_ps[:], xdsT_l[p][:], twT[:, p, :],
                             start=True, stop=True)
            nc.vector.tensor_scalar(M_grp[p // 2][:, p % 2, :], mp_ps[:],
                                    sc_l[p][:], 16.0, MULT, MULT)

    # ============ P5: outT[d, n] = sum_p M_p^T @ ed_p  (+bias) ==========
    with tc.tile_pool(name="psA", bufs=6, space="PSUM") as psA:
        for dc in range(CH):
            osb = ost_pool.tile([128, N], BF16, tag="osb", name="osb")
            for sl in range(NS):
                ops = psA.tile([128, 512], F32, tag="psA", name="ops")
                for u in range(PAIRS // 2):
                    nc.tensor.matmul(
                        ops[:],
                        M_grp[u][:, :, dc * 128:(dc + 1) * 128],
                        ed_grp[u][:, :, sl * 512:(sl + 1) * 512],
                        start=(u == 0), stop=(u == PAIRS // 2 - 1),
                        perf_mode=mybir.MatmulPerfMode.DoubleRow)
                dst = osb[:, sl * 512:(sl + 1) * 512]
                if sl % 2 == 0:
                    nc.scalar.activation(dst, ops[:], Ident,
                                         bias=biasT[:, dc:dc + 1],
                                         scale=1.0 / 16.0)
                else:
                    nc.vector.tensor_scalar(dst, ops[:], 1.0 / 16.0,
                                            biasT[:, dc:dc + 1], MULT, ADD)
                nc.sync.dma_start(
                    out_d[dc * 128:(dc + 1) * 128,
                          sl * 512:(sl + 1) * 512],
                    osb[:, sl * 512:(sl + 1) * 512])


# revision 32
# speedup vs baseline: 1.0595x; 1.0595x over previous
"""Trainium2 Bass kernel for CBSA (cross-block self-attention) module.

Shapes (hardcoded from the problem spec):
  x: [8, 4096, 512], proj_w/to_out_w: [512, 512], step_rep/step_x: [8,1,1],
  to_out_b: [512].  Output: [8, 4096, 512].

Sharding: data-parallel over batch, 1 batch per NeuronCore (8 cores).

Structure:
  - pooling is linear and commutes with the proj GEMM, so pooled x is
    computed on host and rep^T comes from a tiny fp8 on-device GEMM.
  - P1 streams x^T (fp8) in 8 chunks; per chunk: fp8-DoubleRow wT GEMM,
    dots (block-diag rep lhsT), exp -> ed (fp8), and packed transposes:
    a bf16 [128,128] transpose moves a PAIR of adjacent fp8 n-columns,
    halving transpose count; copies run at 2-byte DVE speed.
  - rep_delta contracts the packed pairs via two stride-2 byte-plane fp8
    matmuls per tile; an extra fp8(1,1) column in the packed w tiles
    makes the same matmuls accumulate the softmax row-sums s1 for free.
  - per-pair tail (pairs interleaved step-wise to keep PE fed):
    rep update, pooled self-attention, xds^T computed directly as
    rnat^T @ ed2 (ed2 is symmetric), M = xds^T' @ to_out^T with the
    1/(s1*s2) scale folded into the M copy.
  - P5: out^T[d, n] accumulated over pair-groups with fp8-DoubleRow
    matmuls; bias is a per-partition scalar fused into the PSUM->SBUF
    copy; output is bf16 [C, N], host transposes/casts back.
"""

from contextlib import ExitStack

import numpy as np
import ml_dtypes

import concourse.bass as bass
import concourse.tile as tile
from concourse import bacc, mybir
from concourse import bass_utils

F32 = mybir.dt.float32
BF16 = mybir.dt.bfloat16
FP8 = mybir.dt.float8e4

B = 8
N = 4096
C = 512
HEADS = 8
DH = 64
Q = 64            # pooled tokens
SCALE = DH ** -0.5
NT = N // 128     # 32 token tiles
CH = C // 128     # 4 feature chunks
PAIRS = HEADS // 2  # 4 head pairs
NS = N // 512     # 8 free-dim slices of 512
NCHUNK = 8        # x streamed in 8 chunks of 512 tokens

_CACHE = {}


def _build():
    nc = bacc.Bacc("TRN2", target_bir_lowering=False, debug=False, num_devices=B)

    xT_d = nc.dram_tensor("xT", [128, 2, 2, N], FP8, kind="ExternalInput").ap()
    pwT8_d = nc.dram_tensor("pwT8", [128, 2, 2, C], FP8, kind="ExternalInput").ap()
    dblk_d = nc.dram_tensor("dblk8", [128, PAIRS, 128], FP8,
                            kind="ExternalInput").ap()
    rpT_d = nc.dram_tensor("rpTb", [128, PAIRS, 128], BF16,
                           kind="ExternalInput").ap()
    twT_d = nc.dram_tensor("twT", [128, CH, C], BF16, kind="ExternalInput").ap()
    biasT_d = nc.dram_tensor("biasT", [128, CH], F32, kind="ExternalInput").ap()
    srep_d = nc.dram_tensor("srep", [128, PAIRS], F32, kind="ExternalInput").ap()
    idb_d = nc.dram_tensor("identb", [128, 128], BF16, kind="ExternalInput").ap()
    out_d = nc.dram_tensor("out", [C, N], BF16, kind="ExternalOutput").ap()

    with tile.TileContext(nc) as tc:
        with ExitStack() as ctx:
            _body.ctx = ctx
            _body(tc, nc, xT_d, pwT8_d, dblk_d, rpT_d, twT_d, biasT_d,
                  srep_d, idb_d, out_d)
    nc.compile()
    return nc


def _body(tc, nc, xT_d, pwT8_d, dblk_d, rpT_d, twT_d, biasT_d, srep_d,
          idb_d, out_d):
    Exp = mybir.ActivationFunctionType.Exp
    Ident = mybir.ActivationFunctionType.Identity
    X = mybir.AxisListType.X
    ADD = mybir.AluOpType.add
    MULT = mybir.AluOpType.mult

    ctx = _body.ctx
    const = ctx.enter_context(tc.tile_pool(name="const", bufs=1))
    persist = ctx.enter_context(tc.tile_pool(name="persist", bufs=1))
    xs_pool = ctx.enter_context(tc.tile_pool(name="xstream", bufs=2))
    sm_pool = ctx.enter_context(tc.tile_pool(name="small", bufs=2))
    ost_pool = ctx.enter_context(tc.tile_pool(name="ostage", bufs=2))

    # ---- constants / small inputs (issued before the big x stream) ----
    pwT8 = const.tile([128, 2, 2, C], FP8, tag="pwT8")
    nc.sync.dma_start(pwT8[:], pwT8_d[:])
    dblk_t = const.tile([128, PAIRS, 128], FP8, tag="dblk_t")
    nc.sync.dma_start(dblk_t[:], dblk_d[:])
    rpT_t = const.tile([128, PAIRS, 128], BF16, tag="rpT_t")
    nc.sync.dma_start(rpT_t[:], rpT_d[:])
    # prefetch first two x chunks ahead of the remaining constants
    cols = N // NCHUNK
    xts_list = []
    for ck in range(NCHUNK):
        xts = xs_pool.tile([128, 2, 2, cols], FP8, tag="xs", name=f"xts{ck}")
        xts_list.append(xts)
    for ck in range(3):
        nc.sync.dma_start(xts_list[ck][:],
                          xT_d[:, :, :, ck * cols:(ck + 1) * cols])
    identb = const.tile([128, 128], BF16, tag="identb")
    nc.sync.dma_start(identb[:], idb_d[:])
    srep = const.tile([128, PAIRS], F32, tag="srep")
    nc.sync.dma_start(srep[:], srep_d[:])
    twT = const.tile([128, CH, C], BF16, tag="twT")
    nc.sync.dma_start(twT[:], twT_d[:])
    biasT = const.tile([128, CH], F32, tag="biasT")
    nc.sync.dma_start(biasT[:], biasT_d[:])

    # ---- persistent intermediates ----
    wtb = [persist.tile([128, N], FP8, tag=f"wtb{di}", name=f"wtb{di}")
           for di in range(CH)]
    # packed w natural: bf16 element (n2, d) = fp8 pair (w[2*n2,d], w[2*n2+1,d])
    # per (tile, pair) block of 129 cols: [0:128] w-pairs, col 128 = fp8(1,1)
    # so rep_delta's DR matmul also accumulates s1 = sum_n at[n, q] in col 128.
    w_sb = persist.tile([128, NT // 2, PAIRS, 129], BF16, tag="w_sb")
    ed_grp = [persist.tile([128, 2, N], FP8, tag=f"edg{u}", name=f"edg{u}")
              for u in range(PAIRS // 2)]
    # packed attn^T: bf16 element (n2, q) = fp8 pair (ed[q,2*n2], ed[q,2*n2+1])
    at_list = [persist.tile([128, NT // 2, 128], BF16, tag=f"at{p}",
                            name=f"at{p}")
               for p in range(PAIRS)]
    M_grp = [persist.tile([128, 2, C], FP8, tag=f"Mg{u}", name=f"Mg{u}")
             for u in range(PAIRS // 2)]

    with ExitStack() as phase_ctx:
        psB = phase_ctx.enter_context(
            tc.tile_pool(name="psB", bufs=3, space="PSUM"))
        psC = phase_ctx.enter_context(
            tc.tile_pool(name="psC", bufs=2, space="PSUM"))
        psD = phase_ctx.enter_context(
            tc.tile_pool(name="psD", bufs=3, space="PSUM"))

        ONES_PAIR = float(np.frombuffer(bytes([0x38, 0x38]),
                                        ml_dtypes.bfloat16)[0])
        nc.vector.memset(w_sb[:, :, :, 128:129], ONES_PAIR)

        # rep (pooled queries) is computed on host: dblk = 16*repT
        # block-diag per pair (fp8, dots lhsT), rpT = its natural-layout
        # transpose (bf16, rep update operand).
        dblk = [dblk_t[:, p, :] for p in range(PAIRS)]
        rpT = [rpT_t[:, p, :] for p in range(PAIRS)]

        # ============ P1: streamed wT GEMM + dots + exp + transposes ======
        # wT[d, n] = sum_c pw[d, c] x[n, c]; lhsT = pwT chunk, rhs = xT chunk
        Copy = mybir.ActivationFunctionType.Copy
        for ck in range(NCHUNK):
            c0 = ck * cols
            xts = xts_list[ck]
            if ck >= 3:
                nc.sync.dma_start(xts[:], xT_d[:, :, :, c0:c0 + cols])
            # wT slices for this chunk (fp8 DoubleRow)
            for di in range(CH):
                wps = psB.tile([128, 512], F32, tag="psB", name="wps")
                for g in range(2):
                    nc.tensor.matmul(
                        wps[:], pwT8[:, g, :, di * 128:(di + 1) * 128],
                        xts[:, g, :, :],
                        start=(g == 0), stop=(g == 1),
                        perf_mode=mybir.MatmulPerfMode.DoubleRow)
                dst = wtb[di][:, c0:c0 + cols]
                nc.vector.tensor_scalar_mul(dst, wps[:], 1.0 / 16.0)
            # dots + exp for this chunk's 512-slice, all pairs
            for p in range(PAIRS):
                sl = ck
                dps = psB.tile([128, 512], F32, tag="psB", name="dps")
                nc.tensor.matmul(dps[:], dblk[p][:],
                                 wtb[p][:, sl * 512:(sl + 1) * 512],
                                 start=True, stop=True)
                nc.scalar.activation(
                    ed_grp[p // 2][:, p % 2, sl * 512:(sl + 1) * 512],
                    dps[:], Exp, scale=SCALE / 16.0)
            # packed transposes every 2 chunks: bf16 view pairs adjacent
            # fp8 n-columns, so one [128,128] bf16 transpose covers 2 tiles.
            if ck % 2 == 1:
                t4 = ck // 2            # 4 bf16 n2-tiles per 2-chunk group

                def w_tr(di):
                    wtp = psC.tile([128, 4, 128], BF16, tag="psC", name="wtp")
                    wvv = wtb[di].bitcast(BF16)
                    for j in range(4):
                        nc.tensor.transpose(
                            wtp[:, j, :],
                            wvv[:, (t4 * 4 + j) * 128:(t4 * 4 + j + 1) * 128],
                            identb[:])
                    nc.vector.tensor_copy(
                        w_sb[:, t4 * 4:t4 * 4 + 4, di, 0:128], wtp[:])

                def at_tr(p):
                    atp = psC.tile([128, 4, 128], BF16, tag="psC", name="atp")
                    evv = ed_grp[p // 2][:, p % 2, :].bitcast(BF16)
                    for j in range(4):
                        nc.tensor.transpose(
                            atp[:, j, :],
                            evv[:, (t4 * 4 + j) * 128:(t4 * 4 + j + 1) * 128],
                            identb[:])
                    nc.vector.tensor_copy(
                        at_list[p][:, t4 * 4:t4 * 4 + 4, :], atp[:])

                if ck == NCHUNK - 1:
                    # final chunk: interleave so rep_delta(p) unblocks
                    # progressively in pair order
                    for p in range(PAIRS):
                        w_tr(p)
                        at_tr(p)
                else:
                    for di in range(CH):
                        w_tr(di)
                    for p in range(PAIRS):
                        at_tr(p)

        # ============ P4: pooled attention tail (pairs interleaved) ======
        ssc_l, rnat_l, rnT_l, ed2_l, xds_l, xdsT_l = [], [], [], [], [], []
        rdsb_l = []
        for p in range(PAIRS):
            rd_ps = psD.tile([128, 129], F32, tag="psD", name=f"rd{p}")
            for t in range(NT // 2):
                atv = at_list[p][:, t, :].bitcast(FP8)
                wvv = w_sb[:, t, p, :].bitcast(FP8)
                for j in range(2):
                    nc.tensor.matmul(rd_ps[:], atv[:, j:256:2],
                                     wvv[:, j:258:2],
                                     start=(t == 0 and j == 0),
                                     stop=(t == NT // 2 - 1 and j == 1))
            rdsb = sm_pool.tile([128, 129], F32, tag=f"rdsb{p}",
                                name=f"rdsb{p}")
            nc.vector.tensor_copy(rdsb[:], rd_ps[:])
            rdsb_l.append(rdsb)
        for p in range(PAIRS):
            rc1 = sm_pool.tile([128, 1], F32, tag=f"rc1_{p}", name=f"rc1_{p}")
            nc.vector.reciprocal(rc1[:], rdsb_l[p][:, 128:129])
            ssc = sm_pool.tile([128, 1], F32, tag=f"ssc_{p}", name=f"ssc_{p}")
            nc.vector.tensor_mul(ssc[:], rc1[:], srep[:, p:p + 1])
            ssc_l.append((rc1, ssc))
        for p in range(PAIRS):
            rnat = sm_pool.tile([128, 128], BF16, tag=f"rnat{p}",
                                name=f"rnat{p}")
            nc.vector.memset(rnat[:], 0.0)
            for h in range(2):
                r0, r1 = 64 * h, 64 * (h + 1)
                nc.vector.scalar_tensor_tensor(
                    rnat[r0:r1, r0:r1], rdsb_l[p][r0:r1, r0:r1],
                    ssc_l[p][1][r0:r1, 0:1],
                    rpT[p][r0:r1, r0:r1], MULT, ADD)
            rnat_l.append(rnat)
        for p in range(PAIRS):
            rtp = psC.tile([128, 128], BF16, tag="psC", name=f"rtp{p}")
            nc.tensor.transpose(rtp[:], rnat_l[p][:], identb[:])
            rnT = sm_pool.tile([128, 128], BF16, tag=f"rnT{p}", name=f"rnT{p}")
            nc.vector.tensor_copy(rnT[:], rtp[:])
            rnT_l.append(rnT)
        s2_l = []
        for p in range(PAIRS):
            d2_ps = psD.tile([128, 128], F32, tag="psD", name=f"d2{p}")
            nc.tensor.matmul(d2_ps[:], rnT_l[p][:], rnT_l[p][:],
                             start=True, stop=True)
            ed2 = sm_pool.tile([128, 128], BF16, tag=f"ed2_{p}",
                               name=f"ed2_{p}")
            nc.vector.memset(ed2[:], 0.0)
            s2 = sm_pool.tile([128, 1], F32, tag=f"s2_{p}", name=f"s2_{p}")
            for h in range(2):
                r0, r1 = 64 * h, 64 * (h + 1)
                nc.scalar.activation(ed2[r0:r1, r0:r1], d2_ps[r0:r1, r0:r1],
                                     Exp, scale=SCALE,
                                     accum_out=s2[r0:r1, 0:1])
            ed2_l.append(ed2)
            s2_l.append(s2)
        sc_l = []
        for p in range(PAIRS):
            # xds^T[d, q] = sum_k rnat[k, d] * ed2[q, k]  (ed2 symmetric)
            xt_ps = psD.tile([128, 128], F32, tag="psD", name=f"xt{p}")
            nc.tensor.matmul(xt_ps[:], rnat_l[p][:], ed2_l[p][:],
                             start=True, stop=True)
            xdsT = sm_pool.tile([128, 128], BF16, tag=f"xdsT{p}",
                                name=f"xdsT{p}")
            nc.vector.tensor_copy(xdsT[:], xt_ps[:])
            xdsT_l.append(xdsT)
            rc2 = sm_pool.tile([128, 1], F32, tag=f"rc2_{p}", name=f"rc2_{p}")
            nc.vector.reciprocal(rc2[:], s2_l[p][:])
            sc = sm_pool.tile([128, 1], F32, tag=f"sc_{p}", name=f"sc_{p}")
            nc.vector.tensor_mul(sc[:], ssc_l[p][0][:], rc2[:])
            sc_l.append(sc)
        for p in range(PAIRS):
            mp_ps = psB.tile([128, 512], F32, tag="psB", name=f"mp_ps{p}")
            nc.tensor.matmul(mp_ps[:], xdsT_l[p][:], twT[:, p, :],
                             start=True, stop=True)
            nc.vector.tensor_scalar(M_grp[p // 2][:, p % 2, :], mp_ps[:],
                                    sc_l[p][:], 16.0, MULT, MULT)

    # ============ P5: outT[d, n] = sum_p M_p^T @ ed_p  (+bias) ==========
    with tc.tile_pool(name="psA", bufs=6, space="PSUM") as psA:
        for dc in range(CH):
            osb = ost_pool.tile([128, N], BF16, tag="osb", name="osb")
            for sl in range(NS):
                ops = psA.tile([128, 512], F32, tag="psA", name="ops")
                for u in range(PAIRS // 2):
                    nc.tensor.matmul(
                        ops[:],
                        M_grp[u][:, :, dc * 128:(dc + 1) * 128],
                        ed_grp[u][:, :, sl * 512:(sl + 1) * 512],
                        start=(u == 0), stop=(u == PAIRS // 2 - 1),
                        perf_mode=mybir.MatmulPerfMode.DoubleRow)
                dst = osb[:, sl * 512:(sl + 1) * 512]
                if sl % 2 == 0:
                    nc.scalar.activation(dst, ops[:], Ident,
                                         bias=biasT[:, dc:dc + 1],
                                         scale=1.0 / 16.0)
                else:
                    nc.vector.tensor_scalar(dst, ops[:], 1.0 / 16.0,
                                            biasT[:, dc:dc + 1], MULT, ADD)
                if sl % 2 == 1:
                    nc.sync.dma_start(
                        out_d[dc * 128:(dc + 1) * 128,
                              (sl - 1) * 512:(sl + 1) * 512],
                        osb[:, (sl - 1) * 512:(sl + 1) * 512])


# revision 33
# speedup vs baseline: 1.0923x; 1.0309x over previous
"""Trainium2 Bass kernel for CBSA (cross-block self-attention) module.

Shapes (hardcoded from the problem spec):
  x: [8, 4096, 512], proj_w/to_out_w: [512, 512], step_rep/step_x: [8,1,1],
  to_out_b: [512].  Output: [8, 4096, 512].

Sharding: data-parallel over batch, 1 batch per NeuronCore (8 cores).

Structure:
  - pooling is linear and commutes with the proj GEMM, so pooled x is
    computed on host and rep^T comes from a tiny fp8 on-device GEMM.
  - P1 streams x^T (fp8) in 8 chunks; per chunk: fp8-DoubleRow wT GEMM,
    dots (block-diag rep lhsT), exp -> ed (fp8), and packed transposes:
    a bf16 [128,128] transpose moves a PAIR of adjacent fp8 n-columns,
    halving transpose count; copies run at 2-byte DVE speed.
  - rep_delta contracts the packed pairs via two stride-2 byte-plane fp8
    matmuls per tile; an extra fp8(1,1) column in the packed w tiles
    makes the same matmuls accumulate the softmax row-sums s1 for free.
  - per-pair tail (pairs interleaved step-wise to keep PE fed):
    rep update, pooled self-attention, xds^T computed directly as
    rnat^T @ ed2 (ed2 is symmetric), M = xds^T' @ to_out^T with the
    1/(s1*s2) scale folded into the M copy.
  - P5: out^T[d, n] accumulated over pair-groups with fp8-DoubleRow
    matmuls; bias is a per-partition scalar fused into the PSUM->SBUF
    copy; output is bf16 [C, N], host transposes/casts back.
"""

from contextlib import ExitStack

import numpy as np
import ml_dtypes

import concourse.bass as bass
import concourse.tile as tile
from concourse import bacc, mybir
from concourse import bass_utils

F32 = mybir.dt.float32
BF16 = mybir.dt.bfloat16
FP8 = mybir.dt.float8e4

B = 8
N = 4096
C = 512
HEADS = 8
DH = 64
Q = 64            # pooled tokens
SCALE = DH ** -0.5
NT = N // 128     # 32 token tiles
CH = C // 128     # 4 feature chunks
PAIRS = HEADS // 2  # 4 head pairs
NS = N // 512     # 8 free-dim slices of 512
NCHUNK = 8        # x streamed in 8 chunks of 512 tokens

_CACHE = {}


def _build():
    nc = bacc.Bacc("TRN2", target_bir_lowering=False, debug=False, num_devices=B)

    xT_d = nc.dram_tensor("xT", [128, 2, 2, N], FP8, kind="ExternalInput").ap()
    pwT8_d = nc.dram_tensor("pwT8", [128, 2, 2, C], FP8, kind="ExternalInput").ap()
    dblk_d = nc.dram_tensor("dblk8", [128, PAIRS, 128], FP8,
                            kind="ExternalInput").ap()
    rpT_d = nc.dram_tensor("rpTb", [128, PAIRS, 128], BF16,
                           kind="ExternalInput").ap()
    twT_d = nc.dram_tensor("twT", [128, CH, C], BF16, kind="ExternalInput").ap()
    biasT_d = nc.dram_tensor("biasT", [128, CH], F32, kind="ExternalInput").ap()
    srep_d = nc.dram_tensor("srep", [128, PAIRS], F32, kind="ExternalInput").ap()
    idb_d = nc.dram_tensor("identb", [128, 128], BF16, kind="ExternalInput").ap()
    out_d = nc.dram_tensor("out", [C, N], BF16, kind="ExternalOutput").ap()

    with tile.TileContext(nc) as tc:
        with ExitStack() as ctx:
            _body.ctx = ctx
            _body(tc, nc, xT_d, pwT8_d, dblk_d, rpT_d, twT_d, biasT_d,
                  srep_d, idb_d, out_d)
    nc.compile()
    return nc


def _body(tc, nc, xT_d, pwT8_d, dblk_d, rpT_d, twT_d, biasT_d, srep_d,
          idb_d, out_d):
    Exp = mybir.ActivationFunctionType.Exp
    Ident = mybir.ActivationFunctionType.Identity
    X = mybir.AxisListType.X
    ADD = mybir.AluOpType.add
    MULT = mybir.AluOpType.mult

    ctx = _body.ctx
    const = ctx.enter_context(tc.tile_pool(name="const", bufs=1))
    persist = ctx.enter_context(tc.tile_pool(name="persist", bufs=1))
    xs_pool = ctx.enter_context(tc.tile_pool(name="xstream", bufs=2))
    sm_pool = ctx.enter_context(tc.tile_pool(name="small", bufs=2))
    ost_pool = ctx.enter_context(tc.tile_pool(name="ostage", bufs=2))

    # ---- constants / small inputs (issued before the big x stream) ----
    pwT8 = const.tile([128, 2, 2, C], FP8, tag="pwT8")
    nc.sync.dma_start(pwT8[:], pwT8_d[:])
    dblk_t = const.tile([128, PAIRS, 128], FP8, tag="dblk_t")
    nc.sync.dma_start(dblk_t[:], dblk_d[:])
    # prefetch first x chunks ahead of the remaining constants
    cols = N // NCHUNK
    xts_list = []
    for ck in range(NCHUNK):
        xts = xs_pool.tile([128, 2, 2, cols], FP8, tag="xs", name=f"xts{ck}")
        xts_list.append(xts)
    for ck in range(3):
        nc.sync.dma_start(xts_list[ck][:],
                          xT_d[:, :, :, ck * cols:(ck + 1) * cols])
    identb = const.tile([128, 128], BF16, tag="identb")
    nc.sync.dma_start(identb[:], idb_d[:])
    rpT_t = const.tile([128, PAIRS, 128], BF16, tag="rpT_t")
    nc.sync.dma_start(rpT_t[:], rpT_d[:])
    srep = const.tile([128, PAIRS], F32, tag="srep")
    nc.sync.dma_start(srep[:], srep_d[:])
    twT = const.tile([128, CH, C], BF16, tag="twT")
    nc.sync.dma_start(twT[:], twT_d[:])
    biasT = const.tile([128, CH], F32, tag="biasT")
    nc.sync.dma_start(biasT[:], biasT_d[:])

    # ---- persistent intermediates ----
    wtb = [persist.tile([128, N], FP8, tag=f"wtb{di}", name=f"wtb{di}")
           for di in range(CH)]
    # packed w natural: bf16 element (n2, d) = fp8 pair (w[2*n2,d], w[2*n2+1,d])
    # per (tile, pair) block of 129 cols: [0:128] w-pairs, col 128 = fp8(1,1)
    # so rep_delta's DR matmul also accumulates s1 = sum_n at[n, q] in col 128.
    w_sb = persist.tile([128, NT // 2, PAIRS, 129], BF16, tag="w_sb")
    ed_grp = [persist.tile([128, 2, N], FP8, tag=f"edg{u}", name=f"edg{u}")
              for u in range(PAIRS // 2)]
    # packed attn^T: bf16 element (n2, q) = fp8 pair (ed[q,2*n2], ed[q,2*n2+1])
    at_list = [persist.tile([128, NT // 2, 128], BF16, tag=f"at{p}",
                            name=f"at{p}")
               for p in range(PAIRS)]
    M_grp = [persist.tile([128, 2, C], FP8, tag=f"Mg{u}", name=f"Mg{u}")
             for u in range(PAIRS // 2)]

    with ExitStack() as phase_ctx:
        psB = phase_ctx.enter_context(
            tc.tile_pool(name="psB", bufs=3, space="PSUM"))
        psC = phase_ctx.enter_context(
            tc.tile_pool(name="psC", bufs=2, space="PSUM"))
        psD = phase_ctx.enter_context(
            tc.tile_pool(name="psD", bufs=3, space="PSUM"))

        ONES_PAIR = float(np.frombuffer(bytes([0x38, 0x38]),
                                        ml_dtypes.bfloat16)[0])
        nc.vector.memset(w_sb[:, :, :, 128:129], ONES_PAIR)

        # rep (pooled queries) is computed on host: dblk = 16*repT
        # block-diag per pair (fp8, dots lhsT), rpT = its natural-layout
        # transpose (bf16, rep update operand).
        dblk = [dblk_t[:, p, :] for p in range(PAIRS)]
        rpT = [rpT_t[:, p, :] for p in range(PAIRS)]

        # ============ P1: streamed wT GEMM + dots + exp + transposes ======
        # wT[d, n] = sum_c pw[d, c] x[n, c]; lhsT = pwT chunk, rhs = xT chunk
        Copy = mybir.ActivationFunctionType.Copy
        for ck in range(NCHUNK):
            c0 = ck * cols
            xts = xts_list[ck]
            if ck >= 3:
                nc.sync.dma_start(xts[:], xT_d[:, :, :, c0:c0 + cols])
            # wT slices for this chunk (fp8 DoubleRow)
            for di in range(CH):
                wps = psB.tile([128, 512], F32, tag="psB", name="wps")
                for g in range(2):
                    nc.tensor.matmul(
                        wps[:], pwT8[:, g, :, di * 128:(di + 1) * 128],
                        xts[:, g, :, :],
                        start=(g == 0), stop=(g == 1),
                        perf_mode=mybir.MatmulPerfMode.DoubleRow)
                dst = wtb[di][:, c0:c0 + cols]
                nc.vector.tensor_scalar_mul(dst, wps[:], 1.0 / 16.0)
            # dots + exp for this chunk's 512-slice, all pairs
            for p in range(PAIRS):
                sl = ck
                dps = psB.tile([128, 512], F32, tag="psB", name="dps")
                nc.tensor.matmul(dps[:], dblk[p][:],
                                 wtb[p][:, sl * 512:(sl + 1) * 512],
                                 start=True, stop=True)
                nc.scalar.activation(
                    ed_grp[p // 2][:, p % 2, sl * 512:(sl + 1) * 512],
                    dps[:], Exp, scale=SCALE / 16.0)
            # packed transposes every 2 chunks: bf16 view pairs adjacent
            # fp8 n-columns, so one [128,128] bf16 transpose covers 2 tiles.
            if ck % 2 == 1:
                t4 = ck // 2            # 4 bf16 n2-tiles per 2-chunk group

                def w_tr(di):
                    wtp = psC.tile([128, 4, 128], BF16, tag="psC", name="wtp")
                    wvv = wtb[di].bitcast(BF16)
                    for j in range(4):
                        nc.tensor.transpose(
                            wtp[:, j, :],
                            wvv[:, (t4 * 4 + j) * 128:(t4 * 4 + j + 1) * 128],
                            identb[:])
                    nc.vector.tensor_copy(
                        w_sb[:, t4 * 4:t4 * 4 + 4, di, 0:128], wtp[:])

                def at_tr(p):
                    atp = psC.tile([128, 4, 128], BF16, tag="psC", name="atp")
                    evv = ed_grp[p // 2][:, p % 2, :].bitcast(BF16)
                    for j in range(4):
                        nc.tensor.transpose(
                            atp[:, j, :],
                            evv[:, (t4 * 4 + j) * 128:(t4 * 4 + j + 1) * 128],
                            identb[:])
                    nc.vector.tensor_copy(
                        at_list[p][:, t4 * 4:t4 * 4 + 4, :], atp[:])

                if ck == NCHUNK - 1:
                    # final chunk: interleave so rep_delta(p) unblocks
                    # progressively in pair order
                    for p in range(PAIRS):
                        w_tr(p)
                        at_tr(p)
                else:
                    for di in range(CH):
                        w_tr(di)
                    for p in range(PAIRS):
                        at_tr(p)

        # ============ P4: pooled attention tail (pairs interleaved) ======
        ssc_l, rnat_l, rnT_l, ed2_l, xds_l, xdsT_l = [], [], [], [], [], []
        rdsb_l = []
        for p in range(PAIRS):
            rd_ps = psD.tile([128, 129], F32, tag="psD", name=f"rd{p}")
            for t in range(NT // 2):
                atv = at_list[p][:, t, :].bitcast(FP8)
                wvv = w_sb[:, t, p, :].bitcast(FP8)
                for j in range(2):
                    nc.tensor.matmul(rd_ps[:], atv[:, j:256:2],
                                     wvv[:, j:258:2],
                                     start=(t == 0 and j == 0),
                                     stop=(t == NT // 2 - 1 and j == 1))
            rdsb = sm_pool.tile([128, 129], F32, tag=f"rdsb{p}",
                                name=f"rdsb{p}")
            nc.vector.tensor_copy(rdsb[:], rd_ps[:])
            rdsb_l.append(rdsb)
        for p in range(PAIRS):
            rc1 = sm_pool.tile([128, 1], F32, tag=f"rc1_{p}", name=f"rc1_{p}")
            nc.vector.reciprocal(rc1[:], rdsb_l[p][:, 128:129])
            ssc = sm_pool.tile([128, 1], F32, tag=f"ssc_{p}", name=f"ssc_{p}")
            nc.vector.tensor_mul(ssc[:], rc1[:], srep[:, p:p + 1])
            ssc_l.append((rc1, ssc))
        for p in range(PAIRS):
            rnat = sm_pool.tile([128, 128], BF16, tag=f"rnat{p}",
                                name=f"rnat{p}")
            nc.vector.memset(rnat[:], 0.0)
            for h in range(2):
                r0, r1 = 64 * h, 64 * (h + 1)
                nc.vector.scalar_tensor_tensor(
                    rnat[r0:r1, r0:r1], rdsb_l[p][r0:r1, r0:r1],
                    ssc_l[p][1][r0:r1, 0:1],
                    rpT[p][r0:r1, r0:r1], MULT, ADD)
            rnat_l.append(rnat)
        for p in range(PAIRS):
            rtp = psC.tile([128, 128], BF16, tag="psC", name=f"rtp{p}")
            nc.tensor.transpose(rtp[:], rnat_l[p][:], identb[:])
            rnT = sm_pool.tile([128, 128], BF16, tag=f"rnT{p}", name=f"rnT{p}")
            nc.vector.tensor_copy(rnT[:], rtp[:])
            rnT_l.append(rnT)
        s2_l = []
        for p in range(PAIRS):
            d2_ps = psD.tile([128, 128], F32, tag="psD", name=f"d2{p}")
            nc.tensor.matmul(d2_ps[:], rnT_l[p][:], rnT_l[p][:],
                             start=True, stop=True)
            ed2 = sm_pool.tile([128, 128], BF16, tag=f"ed2_{p}",
                               name=f"ed2_{p}")
            nc.vector.memset(ed2[:], 0.0)
            s2 = sm_pool.tile([128, 1], F32, tag=f"s2_{p}", name=f"s2_{p}")
            for h in range(2):
                r0, r1 = 64 * h, 64 * (h + 1)
                nc.scalar.activation(ed2[r0:r1, r0:r1], d2_ps[r0:r1, r0:r1],
                                     Exp, scale=SCALE,
                                     accum_out=s2[r0:r1, 0:1])
            ed2_l.append(ed2)
            s2_l.append(s2)
        sc_l = []
        for p in range(PAIRS):
            # xds^T[d, q] = sum_k rnat[k, d] * ed2[q, k]  (ed2 symmetric)
            xt_ps = psD.tile([128, 128], F32, tag="psD", name=f"xt{p}")
            nc.tensor.matmul(xt_ps[:], rnat_l[p][:], ed2_l[p][:],
                             start=True, stop=True)
            xdsT = sm_pool.tile([128, 128], BF16, tag=f"xdsT{p}",
                                name=f"xdsT{p}")
            nc.vector.tensor_copy(xdsT[:], xt_ps[:])
            xdsT_l.append(xdsT)
            rc2 = sm_pool.tile([128, 1], F32, tag=f"rc2_{p}", name=f"rc2_{p}")
            nc.vector.reciprocal(rc2[:], s2_l[p][:])
            sc = sm_pool.tile([128, 1], F32, tag=f"sc_{p}", name=f"sc_{p}")
            nc.vector.tensor_mul(sc[:], ssc_l[p][0][:], rc2[:])
            sc_l.append(sc)
        for p in range(PAIRS):
            mp_ps = psB.tile([128, 512], F32, tag="psB", name=f"mp_ps{p}")
            nc.tensor.matmul(mp_ps[:], xdsT_l[p][:], twT[:, p, :],
                             start=True, stop=True)
            nc.vector.tensor_scalar(M_grp[p // 2][:, p % 2, :], mp_ps[:],
                                    sc_l[p][:], 16.0, MULT, MULT)

    # ============ P5: outT[d, n] = sum_p M_p^T @ ed_p  (+bias) ==========
    with tc.tile_pool(name="psA", bufs=6, space="PSUM") as psA:
        for dc in range(CH):
            osb = ost_pool.tile([128, N], BF16, tag="osb", name="osb")
            for sl in range(NS):
                ops = psA.tile([128, 512], F32, tag="psA", name="ops")
                for u in range(PAIRS // 2):
                    nc.tensor.matmul(
                        ops[:],
                        M_grp[u][:, :, dc * 128:(dc + 1) * 128],
                        ed_grp[u][:, :, sl * 512:(sl + 1) * 512],
                        start=(u == 0), stop=(u == PAIRS // 2 - 1),
                        perf_mode=mybir.MatmulPerfMode.DoubleRow)
                dst = osb[:, sl * 512:(sl + 1) * 512]
                if sl % 2 == 0:
                    nc.scalar.activation(dst, ops[:], Ident,
                                         bias=biasT[:, dc:dc + 1],
                                         scale=1.0 / 16.0)
                else:
                    nc.vector.tensor_scalar(dst, ops[:], 1.0 / 16.0,
                                            biasT[:, dc:dc + 1], MULT, ADD)
                if sl % 2 == 1:
                    nc.sync.dma_start(
                        out_d[dc * 128:(dc + 1) * 128,
                              (sl - 1) * 512:(sl + 1) * 512],
                        osb[:, (sl - 1) * 512:(sl + 1) * 512])
